# revision 89
# baseline (speedup 1.0000x reference)
"""Trainium2 Bass kernel: 4-head attention (nn_Attention_75960791598018).

Full inputs in, full outputs out. The batch dim (n=8) is sharded 1:1 across
the 8 NeuronCores (pure data parallelism, no collectives).

v3: fp8(e4m3) DoubleRow matmuls with hi/lo error compensation, software-
pipelined PE emission.

The cost model runs fp8 DoubleRow matmuls at 0.5 cycles/row while
contracting 2x128 per pass (4x the f32r FLOP rate).  Raw fp8 is far too
inaccurate here (every operand's quantization noise lands ~2e-2 on the
output metric: z is a pattern-weighted average over ~2k keys, so noise and
signal shrink by the same sqrt(n_eff) factor and nothing averages out).
Instead every fp8 matmul uses a 3-term hi/lo split
  A@B ~= Ah@Bh + Ah@Bl + Al@Bh   (drops only the ~delta^2 Al@Bl term)
which costs 0.75x the f32r cycles and lands ~2e-3 on the output metric.

Per-core dataflow (x_i: [2048, 1024], contraction dims on partitions):
  host:   xT hi/lo = fp8(8*x.T), W_{Q,K,V} hi/lo = fp8(512*W),
          W_O hi/lo = fp8(512*W_O), cb = b_O + b_V.W_O (exact fold)
  KT/QT[h] = W.T-pairs @ xT-pairs   (fp8 DR 3-term, psum = 4096*q)
             Q evac: t = psum/256 (=16q, ACT f32, +16*b as bias),
             hi = fp8(t) (GpSimd), lo = t - hi (DVE)
             K evac: hi only, single ACT op straight to fp8
  V[h]     = xT-pairs.T @ W-pairs   (fp8 DR 3-term), evac /4096 -> bf16
  scoresT  = Khi-pairs.T @ Q-pairs  (fp8 DR 2-term: Khi*Qhi + Khi*Qlo;
             K-side raw costs ~6e-3 of deterministically-measured error
             for 0.67x scores cycles, psum = 256*s)
  E        = exp(psum/4096)         (ACT -> bf16; scores ~ +-3, no max sub)
  den      = DVE chains (f16 accum = 2x_1p) + GpSimd partition_all_reduce
  zT       = V-tiles.T @ E          (bf16 full-rate: E hi/lo evac would
                                     blow the DVE/ACT budgets)
  z hi/lo  = fp8(256 * zT/den)      (DVE mul + GpSimd copy + DVE sub),
             SBUF-resident for phase C (no DRAM round trip)
  outT     = WO-pairs.T @ z-pairs   (fp8 DR 3-term) * 1/131072 + cb

Scheduling notes (all load-bearing for the measured time):
  - The PE queue is in-order, and exp evacuation (ACT, ~612ns per 512-wide
    tile) is slower than a 3-DR scores group (320ns).  PV matmuls of the
    previous q-group are kept in a backlog and interleaved two-per-scores-
    group so the PE never head-of-line blocks on an ACT-bound PSUM slot.
  - Phase C (out-proj) interleaves with head 3's attention per q-group.
  - DMA descriptor cost = bytes/contiguous-run; every DRAM layout here is
    arranged so each partition's slice is one contiguous run, and traffic
    is spread over three HWDGE queues (x-hi+outT on SP, x-lo on ACT,
    weights on DVE) with nothing on the slow Pool SWDGE path.
  - Power-of-2 scaling is load-bearing: fp8e4m3 min normal is 2^-6; W
    (std 0.02), z (std 0.024) and the lo-residuals would otherwise land
    subnormal.  All rescales ride existing ACT/DVE ops for free.
"""

import os
from contextlib import ExitStack

import numpy as np
import ml_dtypes

import concourse.bass as bass
import concourse.bass_isa as bass_isa
from concourse import bacc
import concourse.mybir as mybir
import concourse.tile as tile
from concourse.bass_utils import run_bass_kernel_spmd

S, D, H, DH = 2048, 1024, 4, 256
P = 128
NT_S = S // P           # 16 s-tiles (keys)
ND_PAIR = D // (2 * P)  # 4 d-tile pairs
QC = 512                # q-chunk = q-group width
NQC = S // QC           # 4
F32 = mybir.dt.float32
F16 = mybir.dt.float16
BF16 = mybir.dt.bfloat16
F8 = mybir.dt.float8e4
NPF8 = ml_dtypes.float8_e4m3
DR = mybir.MatmulPerfMode.DoubleRow
SCALE = 1.0 / 16.0      # 1/sqrt(DH)
N_CORES = 8

SX, SW, SQ, SZ, SWO = 8.0, 512.0, 16.0, 256.0, 512.0

Act = mybir.ActivationFunctionType


def _build():
    nc = bacc.Bacc("TRN2", target_bir_lowering=False, debug=False)
    xthi = nc.dram_tensor("xthi", [NQC, P, ND_PAIR, 2, QC], F8,
                          kind="ExternalInput").ap()
    xtlo = nc.dram_tensor("xtlo", [NQC, P, ND_PAIR, 2, QC], F8,
                          kind="ExternalInput").ap()
    wqkv = {}
    for nm in ("wqh", "wql", "wkh", "wkl", "wvh", "wvl"):
        wqkv[nm] = nc.dram_tensor(nm, [H, P, ND_PAIR, 2, DH], F8,
                                  kind="ExternalInput").ap()
    woh = nc.dram_tensor("woh", [P, H, 2, D], F8, kind="ExternalInput").ap()
    wol = nc.dram_tensor("wol", [P, H, 2, D], F8, kind="ExternalInput").ap()
    bqs = nc.dram_tensor("bqs", [P, H * 2], F32, kind="ExternalInput").ap()
    bks = nc.dram_tensor("bks", [P, H * 2], F32, kind="ExternalInput").ap()
    cbs = nc.dram_tensor("cbs", [P, D // P], F32, kind="ExternalInput").ap()
    # bf16 output: halves the outT DMA tail; host casts back to f32
    # (adds ~1e-3 to the metric, far under the 2e-2 gate)
    outT = nc.dram_tensor("outT", [D, S], BF16, kind="ExternalOutput").ap()

    with tile.TileContext(nc) as tc, ExitStack() as ctx:
        misc = ctx.enter_context(tc.tile_pool(name="misc", bufs=1))
        xp = ctx.enter_context(tc.tile_pool(name="xp", bufs=1))
        wp = ctx.enter_context(tc.tile_pool(name="wp", bufs=1))
        qk = ctx.enter_context(tc.tile_pool(name="qk", bufs=1))
        ep = ctx.enter_context(tc.tile_pool(name="ep", bufs=1))
        vp = ctx.enter_context(tc.tile_pool(name="vp", bufs=1))
        zp = ctx.enter_context(tc.tile_pool(name="zp", bufs=1))
        wk_ = ctx.enter_context(tc.tile_pool(name="wk", bufs=1))
        ps = ctx.enter_context(tc.tile_pool(name="ps", bufs=1, space="PSUM"))

        bq_sb = misc.tile([P, H * 2], F32)
        bk_sb = misc.tile([P, H * 2], F32)
        cb_sb = misc.tile([P, D // P], F32)
        nc.gpsimd.dma_start(out=bq_sb, in_=bqs)
        nc.gpsimd.dma_start(out=bk_sb, in_=bks)
        nc.gpsimd.dma_start(out=cb_sb, in_=cbs)

        def w_tiles(h):
            return {nm: wp.tile([P, ND_PAIR, 2, DH], F8,
                                name=f"{nm}_{h}", tag=f"{nm}{h % 2}")
                    for nm in ("wqh", "wql", "wkh", "wkl", "wvh", "wvl")}

        def w_dma(h, t, names, eng=None):
            # HWDGE queues are SP + ACT only; keep Pool's slow SWDGE path
            # for the tiny bias tiles and spread everything else across both
            for nm in names:
                (eng or nc.scalar).dma_start(out=t[nm], in_=wqkv[nm][h])

        # x: [p, chunk, dpair, half, q] SBUF, chunk-streamed from DRAM
        xh_t = xp.tile([P, NQC, ND_PAIR, 2, QC], F8, name="xh")
        xl_t = xp.tile([P, NQC, ND_PAIR, 2, QC], F8, name="xl")

        # startup DMA order: SP carries x-hi back-to-back, ACT carries the
        # K weights (needed first) then x-lo; the rest of the weights trail
        # on SP.  First K-proj group is ready at ~2.9us.
        wt0 = w_tiles(0)
        w_dma(0, wt0, ("wkh",))
        for c in range(NQC):
            nc.sync.dma_start(out=xh_t[:, c], in_=xthi[c])
            nc.scalar.dma_start(out=xl_t[:, c], in_=xtlo[c])
            if c == 0:
                w_dma(0, wt0, ("wkl",))
            if c == 1:
                # V weights must land before the interleaved V-projection
                # groups reach them (~10us into the PE stream)
                w_dma(0, wt0, ("wvh", "wvl"), eng=nc.sync)
        w_dma(0, wt0, ("wqh", "wql"), eng=nc.sync)
        wo_h = misc.tile([P, H, 2, D], F8, name="woh")
        wo_l = misc.tile([P, H, 2, D], F8, name="wol")
        nc.sync.dma_start(out=wo_h, in_=woh)
        nc.sync.dma_start(out=wo_l, in_=wol)

        # per-head Q/K fp8 tiles: [p(e%128), e-tile, q].  K is stored
        # hi-only: the scores matmul runs 2-term (Khi*Qhi + Khi*Qlo),
        # trading ~6e-3 of (deterministically measured) error for 0.67x
        # scores cycles and a one-op K evacuation.
        qhi = qk.tile([P, 2, S], F8, name="qhi")
        qlo = qk.tile([P, 2, S], F8, name="qlo")
        khi = qk.tile([P, 2, S], F8, name="khi")
        # z, SBUF-resident across heads: [p, h, e-tile, q]
        z_hi = zp.tile([P, H, 2, S], F8, name="zhi")
        z_lo = zp.tile([P, H, 2, S], F8, name="zlo")

        # ---- PE software pipeline: backlog of deferred work items ----
        backlog = []  # (pe_cost_weight, emit_fn)

        def pop_backlog(budget):
            while budget > 0 and backlog:
                w, f = backlog.pop(0)
                f()
                budget -= w

        def dr3(pp, ah, al, bh, bl, npair):
            # hi*hi products first: the lo operands may still be in flight
            # (DMA or evac chain) when the group starts
            n3 = 3 * npair
            i = 0
            for a, b in ((ah, bh), (ah, bl), (al, bh)):
                for j in range(npair):
                    nc.tensor.matmul(pp, a(j), b(j), start=(i == 0),
                                     stop=(i == n3 - 1), perf_mode=DR)
                    i += 1

        def proj_qk_group(h, wt, isq, c, et):
            """One Q/K projection psum group: chunk c, e-tile et."""
            csl = slice(c * QC, (c + 1) * QC)
            esl = slice(et * P, (et + 1) * P)
            wh, wl = (wt["wqh"], wt["wql"]) if isq else (wt["wkh"], wt["wkl"])
            b_sb = bq_sb if isq else bk_sb
            pp = ps.tile([P, QC], F32, name=f"pp{h}_{isq}_{et}_{c}",
                         tag="pp", bufs=3)
            dr3(pp,
                lambda j: wh[:, j, :, esl], lambda j: wl[:, j, :, esl],
                lambda j: xh_t[:, c, j], lambda j: xl_t[:, c, j], ND_PAIR)
            bias = b_sb[:, h * 2 + et:h * 2 + et + 1]
            if isq:
                tf = wk_.tile([P, QC], F32, name=f"tf{h}_{et}_{c}",
                              tag="tf", bufs=4)
                nc.scalar.activation(out=tf, in_=pp, func=Act.Identity,
                                     scale=SQ / (SX * SW), bias=bias)
                nc.gpsimd.tensor_copy(out=qhi[:, et, csl], in_=tf)
                nc.vector.tensor_sub(qlo[:, et, csl], tf, qhi[:, et, csl])
            else:
                # K hi-only: single ACT evac straight to fp8
                nc.scalar.activation(out=khi[:, et, csl], in_=pp,
                                     func=Act.Identity,
                                     scale=SQ / (SX * SW), bias=bias)

        def proj_v_group(h, wt, vt, si):
            c, off = si // 4, si % 4
            ssl = slice(off * P, (off + 1) * P)
            pv = ps.tile([P, DH], F32, name=f"pv{h}_{si}", tag="pp", bufs=3)
            dr3(pv,
                lambda j: xh_t[:, c, j, :, ssl], lambda j: xl_t[:, c, j, :, ssl],
                lambda j: wt["wvh"][:, j], lambda j: wt["wvl"][:, j], ND_PAIR)
            nc.scalar.activation(out=vt[si], in_=pv, func=Act.Identity,
                                 scale=1.0 / (SX * SW))

        def sc_group(h, g, kt, es):
            csl = slice(g * QC, (g + 1) * QC)
            ksl = slice(kt * P, (kt + 1) * P)
            psc = ps.tile([P, QC], F32, name=f"sc{h}_{g}_{kt}", tag="sc",
                          bufs=3)
            nc.tensor.matmul(psc, khi[:, :, ksl], qhi[:, :, csl],
                             start=True, stop=False, perf_mode=DR)
            nc.tensor.matmul(psc, khi[:, :, ksl], qlo[:, :, csl],
                             start=False, stop=True, perf_mode=DR)
            nc.scalar.activation(out=es[:, kt], in_=psc, func=Act.Exp,
                                 scale=SCALE / (SQ * SQ))

        def den_add(h, g, kt, es, da):
            """One den-chain step, emitted right after its exp so the chain
            finishes with the stretch instead of ~6us after it (the rbs ->
            zf -> z-evac chain is what stalls PV slots and phase C)."""
            hn = NT_S // 2
            j, k = kt // hn, kt % hn
            if k == 1:
                nc.vector.tensor_add(da[j], es[:, j * hn], es[:, j * hn + 1])
            elif k >= 2:
                nc.vector.tensor_add(da[j], da[j], es[:, kt])

        def den_finish(h, g, da):
            """Combine + partition reduce -> rbs = SZ/den."""
            daf = wk_.tile([P, QC], F32, name=f"daf{h}_{g}", tag="daf", bufs=2)
            nc.vector.tensor_add(daf, da[0], da[1])
            denb = wk_.tile([P, QC], F32, name=f"db{h}_{g}", tag="denb",
                            bufs=2)
            nc.gpsimd.partition_all_reduce(denb, daf, channels=P,
                                           reduce_op=bass_isa.ReduceOp.add)
            rb = wk_.tile([P, QC], F32, name=f"rb{h}_{g}", tag="rb", bufs=2)
            nc.vector.reciprocal(out=rb, in_=denb)
            rbs = wk_.tile([P, QC], F32, name=f"rs{h}_{g}", tag="rbs", bufs=2)
            nc.vector.tensor_scalar_mul(rbs, rb, SZ)
            return rbs

        def push_pv(h, g, es, vt, rbs):
            """Defer PV + z-evac as backlog items (one matmul each)."""
            gsl = slice(g * QC, (g + 1) * QC)
            for et in range(2):
                pz = ps.tile([P, QC], F32, name=f"pz{h}_{g}_{et}", tag="pz",
                             bufs=2)

                def mk(kt, pz=pz, et=et):
                    def item():
                        nc.tensor.matmul(pz, vt[kt][:, et * P:(et + 1) * P],
                                         es[:, kt], start=(kt == 0),
                                         stop=(kt == NT_S - 1))
                        if kt == NT_S - 1:
                            zf = wk_.tile([P, QC], F32, name=f"zf{h}_{g}_{et}",
                                          tag=f"zf{et}", bufs=2)
                            nc.vector.tensor_mul(zf, pz, rbs)
                            nc.gpsimd.tensor_copy(out=z_hi[:, h, et, gsl],
                                                  in_=zf)
                            nc.vector.tensor_sub(z_lo[:, h, et, gsl], zf,
                                                 z_hi[:, h, et, gsl])
                    return item

                for kt in range(NT_S):
                    backlog.append((1, mk(kt)))

        def out_group(g, dt):
            csl = slice(g * QC, (g + 1) * QC)
            dsl = slice(dt * P, (dt + 1) * P)
            po = ps.tile([P, QC], F32, name=f"po{g}_{dt}", tag="pp", bufs=3)
            dr3(po,
                lambda j: wo_h[:, j, :, dsl], lambda j: wo_l[:, j, :, dsl],
                lambda j: z_hi[:, j, :, csl], lambda j: z_lo[:, j, :, csl], H)
            ot = wk_.tile([P, QC], BF16, name=f"ot{g}_{dt}", tag="ot", bufs=6)
            nc.scalar.activation(out=ot, in_=po, func=Act.Identity,
                                 scale=1.0 / (SZ * SWO),
                                 bias=cb_sb[:, dt:dt + 1])
            nc.sync.dma_start(out=outT[dsl, csl], in_=ot)

        # ==================== head loop ====================
        wt = wt0
        for h in range(H):
            vt = [vp.tile([P, DH], BF16, name=f"v{h}_{s}", tag=f"v{s}")
                  for s in range(NT_S)]
            # K and V projections interleaved: at h0 the order KP0 KP1 VP0-3
            # KP2 VP4-7 KP3 matches each tensor's DMA arrival time
            for c in range(NQC):
                for et in range(2):
                    proj_qk_group(h, wt, False, c, et)
                    pop_backlog(4)
                for si in ((4, 5, 6, 7) if c == 2 else
                           (8, 9, 10, 11, 12, 13, 14, 15) if c == 3 else
                           (0, 1, 2, 3) if c == 1 else ()):
                    if (h == 0 and si >= 12) or (h == H - 1 and si >= 8):
                        # defer as scores-stretch filler: the backlog is
                        # otherwise empty at g0 of the first head (nothing
                        # pushed yet) and of the last head (PV of the prior
                        # head drained during this head's K projection);
                        # without it the PE outruns the Q evac chain
                        backlog.append(
                            (3, lambda si=si: proj_v_group(h, wt, vt, si)))
                    elif h == 0 and si >= 8:
                        pass  # emitted after the bootstrap Q projection
                    else:
                        proj_v_group(h, wt, vt, si)
                        pop_backlog(0 if h == H - 1 else 1)
            if h + 1 < H:
                # prefetch on SP (idle mid-run); the ACT queue must stay
                # clear for the attention evacuation chain
                wt_next = w_tiles(h + 1)
                w_dma(h + 1, wt_next,
                      ("wkh", "wkl", "wvh", "wvl", "wqh", "wql"),
                      eng=nc.sync)

            if h == 0:
                # bootstrap: the very first Q projection, followed by the
                # un-deferred V-projection groups so its ACT->Pool->DVE evac
                # chain hides under them before the first scores stretch
                for et in range(2):
                    proj_qk_group(0, wt, True, 0, et)
                for si in (8, 9, 10, 11):
                    proj_v_group(h, wt, vt, si)
            for g in range(NQC):            # attention, q-group == chunk
                es = ep.tile([P, NT_S, QC], BF16, name=f"e{h}_{g}",
                             tag="es", bufs=2)
                da = [wk_.tile([P, QC], F16, name=f"da{h}_{g}_{j}",
                               tag=f"da{j}", bufs=2) for j in range(2)]
                for kt in range(NT_S):
                    sc_group(h, g, kt, es)
                    den_add(h, g, kt, es, da)
                    # at the last head, drain faster so the z evacuation
                    # chain finishes before the out-proj stretch needs it
                    pop_backlog(3 if h == H - 1 else 2)
                    if kt == 5 and not (h == H - 1 and g >= 1):
                        # inject the NEXT q-group's Q projection here so its
                        # ACT->Pool->DVE evac chain finishes before its
                        # scores stretch begins (kills the boundary stall).
                        # At h3 with an out-proj stretch running, the
                        # injection moves there instead (the tf evacuation
                        # otherwise queues behind exp evacs on ACT and
                        # delays PSUM slot recycling).
                        nh, ng = (h, g + 1) if g + 1 < NQC else (h + 1, 0)
                        if nh < H:
                            nwt = wt if nh == h else wt_next
                            for et in range(2):
                                proj_qk_group(nh, nwt, True, ng, et)
                                pop_backlog(2)
                rbs = den_finish(h, g, da)
                push_pv(h, g, es, vt, rbs)
                if h == H - 1 and g >= 1:
                    # next q-group's Q projection first: its 2.56us of PE
                    # also delays OUT(g-1) past the z(g-1) evacuation
                    if g + 1 < NQC:
                        for et in range(2):
                            proj_qk_group(h, wt, True, g + 1, et)
                            pop_backlog(2)
                    # out-proj for the previous q-group; only the last
                    # q-group's PV drains here (earlier ones must survive
                    # as filler for the next scores stretch)
                    for dt in range(D // P):
                        out_group(g - 1, dt)
                        if g == NQC - 1:
                            # drain PV(g3) early in the stretch so its z
                            # evacuation lands before OUT(g3) begins
                            pop_backlog(6)

            wt = wt_next if h + 1 < H else wt

        pop_backlog(10 ** 9)                # drain PV of (h3, g3)
        for dt in range(D // P):
            out_group(NQC - 1, dt)

    nc.compile()
    return nc


_CACHE = {}


def _get_nc():
    if "nc" not in _CACHE:
        _CACHE["nc"] = _build()
    return _CACHE["nc"]


LAST_RESULTS = None


def _hilo(x, s):
    hi = (s * x).astype(NPF8)
    lo = (s * x - hi.astype(np.float32)).astype(NPF8)
    return hi, lo


def kernel(**inputs) -> np.ndarray:
    x = np.ascontiguousarray(np.asarray(inputs["normalized_resid_pre"],
                                        dtype=np.float32))
    n = x.shape[0]
    assert x.shape == (N_CORES, S, D), x.shape
    w_q = np.asarray(inputs["W_Q"], np.float32)
    w_k = np.asarray(inputs["W_K"], np.float32)
    w_v = np.asarray(inputs["W_V"], np.float32)
    w_o = np.asarray(inputs["W_O"], np.float32)
    b_q = np.asarray(inputs["b_Q"], np.float32)
    b_k = np.asarray(inputs["b_K"], np.float32)
    b_v = np.asarray(inputs["b_V"], np.float32)
    b_o = np.asarray(inputs["b_O"], np.float32)

    for w in (w_q, w_k, w_v, w_o):
        assert np.abs(SW * w).max() < 200.0, "fp8 overflow risk in W"

    def lay_w(w):  # [D, DH] -> [P, ND_PAIR, 2, DH]
        return np.ascontiguousarray(
            w.reshape(ND_PAIR, 2, P, DH).transpose(2, 0, 1, 3))

    base = {}
    for nm, w in (("wq", w_q), ("wk", w_k), ("wv", w_v)):
        hi = np.empty((H, P, ND_PAIR, 2, DH), NPF8)
        lo = np.empty((H, P, ND_PAIR, 2, DH), NPF8)
        for h in range(H):
            a, b = _hilo(w[h], SW)
            hi[h] = lay_w(a)
            lo[h] = lay_w(b)
        base[nm + "h"] = hi
        base[nm + "l"] = lo
    oh, ol = _hilo(w_o, SWO)
    base["woh"] = np.ascontiguousarray(
        oh.reshape(H, 2, P, D).transpose(2, 0, 1, 3))
    base["wol"] = np.ascontiguousarray(
        ol.reshape(H, 2, P, D).transpose(2, 0, 1, 3))
    base["bqs"] = np.ascontiguousarray((SQ * b_q).reshape(H * 2, P).T)
    base["bks"] = np.ascontiguousarray((SQ * b_k).reshape(H * 2, P).T)
    cb = b_o + np.tensordot(b_v, w_o, axes=([0, 1], [0, 1])).astype(np.float32)
    base["cbs"] = np.ascontiguousarray(cb.reshape(D // P, P).T)

    nc = _get_nc()
    in_maps = []
    for i in range(n):
        xt = np.ascontiguousarray(x[i].T)  # [D, S]
        hi, lo = _hilo(xt, SX)

        def lay_x(a):  # [D, S] -> [NQC, P, ND_PAIR, 2, QC]
            return np.ascontiguousarray(
                a.reshape(ND_PAIR, 2, P, NQC, QC).transpose(3, 2, 0, 1, 4))

        m = dict(base)
        m["xthi"] = lay_x(hi)
        m["xtlo"] = lay_x(lo)
        in_maps.append(m)
    trace = os.environ.get("KERNEL_TRACE", "0") == "1"
    res = run_bass_kernel_spmd(nc, in_maps, core_ids=list(range(N_CORES)),
                               trace=trace)
    global LAST_RESULTS
    LAST_RESULTS = res
    return np.stack([res.results[i]["outT"].T.astype(np.float32)
                     for i in range(n)], axis=0)
